# revision 21
# baseline (speedup 1.0000x reference)
"""Trainium2 Bass kernel for nn_BaseShiftNet.

reference(x, dates, thetas) returns (out, x_pairs):
  - out    = per-frame translation-only bilinear grid_sample of raw x
             (zeros padding), thetas in pixels.
  - x_pairs[:, :, 0] = grayscale(ITU-R 601) of per-(n,t,c) min/max
             normalized x (channels 0:3)
  - x_pairs[:, :, 1] = the grayscale of the frame whose date is closest
             to day 182, broadcast over t.

Strategy (pure data parallel, 8 cores, 12 frames each; each core's
frames all belong to one sample n):
  - vertical shift+fraction folded into a host-built banded matrix MvT
    per frame (runtime input -> same compiled graph on all cores),
    applied as a TensorE matmul.
  - horizontal integer shift via runtime register column offsets
    (values_load + bass.ds) into a zero-margined SBUF copy of the
    frame; fractional lerp = one ACT op + one DVE scalar_tensor_tensor
    with per-frame runtime scalars.
  - min/max reduced on DVE (free axis) then gpsimd partition_all_reduce;
    normalization scalars computed on-chip; grayscale via ACT
    Identity(scale=a0, bias=b) + two DVE scalar_tensor_tensor ops.
  - compute in bf16 (inputs cast during DMA), accumulation f32.
"""

import sys
import types
import numpy as np

sys.path.insert(0, "/opt/trn_rl_repo")

import ml_dtypes  # noqa: E402

REF_DAY = 182
EPS = 1e-8
GRAY_W = (0.299, 0.587, 0.114)

N, T, C, H, W = 4, 24, 4, 256, 256
NCORES = 8
FPC = (N * T) // NCORES          # frames per core = 12
NF = FPC + 1                     # stats slots: frames + ref


# ---------------------------------------------------------------------------
# workarounds for this environment
# ---------------------------------------------------------------------------

def _install_tile_drain_patch():
    """This walrus build rejects >1 sync-wait on the Tile exit drain
    (CTRL) instruction; spread the waits across one-wait sync nops."""
    import concourse.tile as tile
    from concourse import mybir
    from concourse.vector_clock import ScopedClock

    if getattr(tile.TileContext, "_drain_patch_installed", False):
        return

    def _patched(self, tick_clock, wait_clock):
        nc = self.nc
        drain_inst = nc.sync.drain()
        wait_clock.add_sem_waits(
            drain_inst.ins, ScopedClock({None: tick_clock.global_clock})
        )
        si = drain_inst.ins.sync_info
        if si is not None and len(si.on_wait) > 0:
            waits = list(si.on_wait)
            drain_inst.ins.sync_info = mybir.SyncInfo(
                on_wait=[], on_update=list(si.on_update)
            )
            for w in waits:
                n = nc.sync.nop()
                n.ins.sync_info = mybir.SyncInfo(on_wait=[w], on_update=[])
        nc.all_engine_barrier()
        popped = nc._tile_sem_poison_stack.pop()
        assert popped is self._sem_poison
        nc.clear_and_free_semaphores(list(self.sems.allocated().values()))
        nc.all_engine_barrier()

    tile.TileContext._drain_and_barrier = _patched
    tile.TileContext._drain_patch_installed = True


def _split_multi_wait_instructions(nc, max_waits=1):
    """This walrus build allows at most one sync-wait per instruction.
    Hoist extra waits onto injected same-engine NOPs placed just before.
    NOPs are created via the engine API (so they are registered for the
    simulator/runtime) and then repositioned by rewriting block lists."""
    from concourse import mybir

    # snapshot all block instruction lists before creating any nops
    blocks = [b for f in nc.m.functions for b in f.blocks]
    snapshots = [list(b.instructions) for b in blocks]

    ctr = 0
    new_lists = []
    for insts in snapshots:
        out = []
        for inst in insts:
            si = inst.sync_info
            if si is not None and len(si.on_wait) > max_waits:
                waits = list(si.on_wait)
                for w in waits[:-max_waits]:
                    ctr += 1
                    nop = nc.engines[inst.engine].nop().ins
                    nop.sync_info = mybir.SyncInfo(on_wait=[w], on_update=[])
                    out.append(nop)
                inst.sync_info = mybir.SyncInfo(
                    on_wait=waits[-max_waits:], on_update=list(si.on_update)
                )
            out.append(inst)
        new_lists.append(out)

    # overwrite lists (this also drops the auto-appended nop copies)
    for b, out in zip(blocks, new_lists):
        b.instructions = out
    return ctr


# ---------------------------------------------------------------------------
# device graph
# ---------------------------------------------------------------------------

_GRAPH_CACHE = {}
_LAST_IN_MAPS = None


def _build_graph(M):
    """Build the SPMD Bass graph for margin M. Returns (nc, names)."""
    import concourse.bass as bass
    import concourse.tile as tile
    from concourse import mybir

    _install_tile_drain_patch()

    WB = 2 * M + 258          # B2 buffer width: [0..M) zeros, image, zeros
    SPAD = 40                 # padded stats count (NF*3 = 39 -> 40)
    f32 = mybir.dt.float32
    bf16 = mybir.dt.bfloat16
    i32 = mybir.dt.int32

    nc = bass.Bass()

    xs_ext = nc.declare_dram_parameter("xs", [FPC, C, H, W], f32, isOutput=False)
    ref_ext = nc.declare_dram_parameter("ref", [3, H, W], f32, isOutput=False)
    mvt_ext = nc.declare_dram_parameter("mvt", [128, FPC, 2, 256], bf16, isOutput=False)
    fr_ext = nc.declare_dram_parameter("fracs", [128, 2 * FPC], f32, isOutput=False)
    offs_ext = nc.declare_dram_parameter("offs", [1, 2 * FPC], i32, isOutput=False)
    ident_ext = nc.declare_dram_parameter("ident", [128, 128], f32, isOutput=False)
    gwc_ext = nc.declare_dram_parameter("gwc", [SPAD, 1], f32, isOutput=False)

    out_ext = nc.declare_dram_parameter("out", [FPC, C, H, W], f32, isOutput=True)
    slc_ext = nc.declare_dram_parameter("slc", [FPC, H, W], f32, isOutput=True)
    rslc_ext = nc.declare_dram_parameter("rslc", [H, W], f32, isOutput=True)

    ACT = mybir.EngineType.Activation
    DVE = mybir.EngineType.DVE
    AF = mybir.ActivationFunctionType
    OP = mybir.AluOpType

    with tile.TileContext(nc) as tc:
        with (
            tc.tile_pool(name="const", bufs=1) as cpool,
            tc.tile_pool(name="b2", bufs=FPC) as b2pool,
            tc.tile_pool(name="refx", bufs=1) as refpool,
            tc.tile_pool(name="work", bufs=3) as wpool,
            tc.tile_pool(name="outb", bufs=3) as opool,
            tc.tile_pool(name="gray", bufs=4) as gpool,
            tc.tile_pool(name="stats", bufs=1) as spool,
            tc.tile_pool(name="psum", bufs=2, space="PSUM") as ppool,
        ):
            # ---- constants in ----
            mvt_sb = cpool.tile([128, FPC, 2, 256], bf16, tag="mvt")
            nc.sync.dma_start(mvt_sb[:], mvt_ext[:])
            fr_sb = cpool.tile([128, 2 * FPC], f32, tag="fr")
            nc.sync.dma_start(fr_sb[:], fr_ext[:])
            offs_sb = cpool.tile([1, 2 * FPC], i32, tag="offs")
            nc.sync.dma_start(offs_sb[:], offs_ext[:])
            ident_sb = cpool.tile([128, 128], f32, tag="ident")
            nc.sync.dma_start(ident_sb[:], ident_ext[:])
            gwc_sb = cpool.tile([SPAD, 1], f32, tag="gwc")
            nc.sync.dma_start(gwc_sb[:], gwc_ext[:])
            ones_sb = cpool.tile([1, 128], f32, tag="ones")
            nc.vector.memset(ones_sb[:], 1.0)

            # ---- stats tiles (flat SPAD layout; slot s = f*3 + c) ----
            minp = spool.tile([128, SPAD], f32, tag="minp")
            maxp = spool.tile([128, SPAD], f32, tag="maxp")
            nc.vector.memset(minp[:, NF * 3:SPAD], 0.0)
            nc.vector.memset(maxp[:, NF * 3:SPAD], 1.0)

            # ---- ref frame: load + stats partials ----
            refx = refpool.tile([128, 3, 2, 256], bf16, tag="refx")
            nc.gpsimd.dma_start(refx[:], ref_ext.rearrange("c (h p) w -> p c h w", p=128))
            rtm = wpool.tile([128, 3, 256], bf16, tag="ttmp")
            nc.vector.tensor_tensor(rtm[:], refx[:, :, 0, :], refx[:, :, 1, :], op=OP.min)
            nc.vector.tensor_reduce(
                minp[:, 3 * FPC:3 * FPC + 3], rtm[:], axis=mybir.AxisListType.X, op=OP.min
            )
            rtx = wpool.tile([128, 3, 256], bf16, tag="ttmp")
            nc.vector.tensor_tensor(rtx[:], refx[:, :, 0, :], refx[:, :, 1, :], op=OP.max)
            nc.vector.tensor_reduce(
                maxp[:, 3 * FPC:3 * FPC + 3], rtx[:], axis=mybir.AxisListType.X, op=OP.max
            )

            # ---- per-frame main pipeline ----
            b2_tiles = []
            for f in range(FPC):
                b2 = b2pool.tile([128, C, 2, WB], bf16, tag="b2")
                b2_tiles.append(b2)
                # zero margins (gpsimd; the image DMA fills the middle)
                nc.gpsimd.memset(b2[:, :, :, 0:M], 0.0)
                nc.gpsimd.memset(b2[:, :, :, M + 256:WB], 0.0)
                nc.gpsimd.dma_start(
                    b2[:, :, :, M:M + 256],
                    xs_ext[f].rearrange("c (h p) w -> p c h w", p=128),
                )

                # stats partials (channels 0:3)
                tmn = wpool.tile([128, 3, 256], bf16, tag="ttmp")
                nc.vector.tensor_tensor(
                    tmn[:], b2[:, 0:3, 0, M:M + 256], b2[:, 0:3, 1, M:M + 256], op=OP.min
                )
                nc.vector.tensor_reduce(
                    minp[:, 3 * f:3 * f + 3], tmn[:], axis=mybir.AxisListType.X, op=OP.min
                )
                tmx = wpool.tile([128, 3, 256], bf16, tag="ttmp")
                nc.vector.tensor_tensor(
                    tmx[:], b2[:, 0:3, 0, M:M + 256], b2[:, 0:3, 1, M:M + 256], op=OP.max
                )
                nc.vector.tensor_reduce(
                    maxp[:, 3 * f:3 * f + 3], tmx[:], axis=mybir.AxisListType.X, op=OP.max
                )

                # horizontal lerp with runtime column offset
                off0 = nc.values_load(
                    offs_sb[0:1, 2 * f:2 * f + 1], engines=(DVE,),
                    min_val=0, max_val=2 * M + 1, skip_runtime_bounds_check=True,
                )
                off1 = nc.values_load(
                    offs_sb[0:1, 2 * f + 1:2 * f + 2], engines=(ACT,),
                    min_val=0, max_val=2 * M + 1, skip_runtime_bounds_check=True,
                )
                th = wpool.tile([128, C, 2, 256], bf16, tag="th")
                nc.scalar.activation(
                    th[:], b2[:, :, :, bass.ds(off1, 256)], AF.Copy,
                    scale=fr_sb[:, 2 * f + 1:2 * f + 2],
                )
                wt = wpool.tile([128, C, 2, 256], bf16, tag="wt")
                nc.vector.scalar_tensor_tensor(
                    wt[:], b2[:, :, :, bass.ds(off0, 256)],
                    fr_sb[:, 2 * f:2 * f + 1], th[:],
                    op0=OP.mult, op1=OP.add,
                )

                # vertical shift+lerp via banded matmul
                ps = ppool.tile([128, 2, C, 256], f32, tag="ps")
                for mh in range(2):
                    for kh in range(2):
                        for cp in range(2):
                            nc.tensor.matmul(
                                ps[:, mh, 2 * cp:2 * cp + 2, :],
                                mvt_sb[:, f, kh, mh * 128:(mh + 1) * 128],
                                wt[:, 2 * cp:2 * cp + 2, kh, :],
                                start=(kh == 0), stop=(kh == 1),
                            )
                ob = opool.tile([128, 2, C, 256], f32, tag="ob")
                out_dst = out_ext[f].rearrange("c (m p) w -> m p c w", p=128)
                for mh in range(2):
                    nc.scalar.copy(ob[:, mh], ps[:, mh])
                    nc.sync.dma_start(out_dst[mh], ob[:, mh])

            # ---- global stats: transpose -> free-axis reduce -> math on
            # SPAD partitions -> transpose+ones-matmul broadcast back ----
            pmnT = ppool.tile([SPAD, 128], f32, tag="ps")
            nc.tensor.transpose(pmnT[:], minp[:], ident_sb[:])
            mn1 = spool.tile([SPAD, 1], f32, tag="mn1")
            nc.vector.tensor_reduce(mn1[:], pmnT[:], axis=mybir.AxisListType.X, op=OP.min)
            pmxT = ppool.tile([SPAD, 128], f32, tag="ps")
            nc.tensor.transpose(pmxT[:], maxp[:], ident_sb[:])
            mx1 = spool.tile([SPAD, 1], f32, tag="mx1")
            nc.vector.tensor_reduce(mx1[:], pmxT[:], axis=mybir.AxisListType.X, op=OP.max)

            # scalar math on SPAD partitions
            den1 = spool.tile([SPAD, 1], f32, tag="den1")
            nc.vector.scalar_tensor_tensor(
                den1[:], mx1[:], 1.0, mn1[:], op0=OP.mult, op1=OP.subtract
            )
            nc.vector.tensor_scalar_add(den1[:], den1[:], EPS)
            rc1 = spool.tile([SPAD, 1], f32, tag="rc1")
            nc.vector.reciprocal(rc1[:], den1[:])
            # a = gray_w * recip; transpose a-col and min-col to rows at
            # partition 0, then ones-matmul broadcast to all partitions
            acol = spool.tile([SPAD, 1], f32, tag="acol")
            nc.vector.tensor_tensor(acol[:], rc1[:], gwc_sb[:], op=OP.mult)
            rows = spool.tile([1, 2, SPAD], f32, tag="rows")
            for r, col in ((0, acol), (1, mn1)):
                rT = ppool.tile([1, SPAD], f32, tag="ps")
                nc.tensor.transpose(rT[:], col[:], ident_sb[0:SPAD, 0:SPAD])
                nc.scalar.copy(rows[:, r, :], rT[:])
            bc = ppool.tile([128, 2, SPAD], f32, tag="ps")
            for r in range(2):
                nc.tensor.matmul(
                    bc[:, r, :], ones_sb[:], rows[:, r, :],
                    start=True, stop=True,
                )
            acoef = spool.tile([128, SPAD], f32, tag="acoef")
            nc.scalar.copy(acoef[:], bc[:, 0, :])
            mnb = spool.tile([128, SPAD], f32, tag="mnb")
            nc.scalar.copy(mnb[:], bc[:, 1, :])

            # b = -sum_c a_c * min_c  per stats-frame
            prod = spool.tile([128, SPAD], f32, tag="prod")
            nc.vector.tensor_tensor(prod[:], acoef[:], mnb[:], op=OP.mult)
            bcoef = spool.tile([128, NF], f32, tag="bcoef")
            nc.vector.tensor_reduce(
                bcoef[:],
                prod[:, 0:NF * 3].rearrange("p (f c) -> p f c", c=3),
                axis=mybir.AxisListType.X, op=OP.add,
            )
            nc.vector.tensor_scalar_mul(bcoef[:], bcoef[:], -1.0)

            # ---- grayscale outputs ----
            def gray_chain(src_c0, src_c1, src_c2, fidx, dst_dram):
                g1 = gpool.tile([128, 2, 256], bf16, tag="g1")
                nc.scalar.activation(
                    g1[:], src_c0, AF.Identity,
                    scale=acoef[:, 3 * fidx:3 * fidx + 1],
                    bias=bcoef[:, fidx:fidx + 1],
                )
                g2 = gpool.tile([128, 2, 256], bf16, tag="g2")
                nc.vector.scalar_tensor_tensor(
                    g2[:], src_c1, acoef[:, 3 * fidx + 1:3 * fidx + 2], g1[:],
                    op0=OP.mult, op1=OP.add,
                )
                g3 = gpool.tile([128, 2, 256], bf16, tag="g3")
                nc.vector.scalar_tensor_tensor(
                    g3[:], src_c2, acoef[:, 3 * fidx + 2:3 * fidx + 3], g2[:],
                    op0=OP.mult, op1=OP.add,
                )
                nc.gpsimd.dma_start(
                    dst_dram.rearrange("(h p) w -> p h w", p=128), g3[:]
                )

            gray_chain(
                refx[:, 0, :, :], refx[:, 1, :, :], refx[:, 2, :, :],
                FPC, rslc_ext,
            )
            for f in range(FPC):
                b2 = b2_tiles[f]
                gray_chain(
                    b2[:, 0, :, M:M + 256], b2[:, 1, :, M:M + 256],
                    b2[:, 2, :, M:M + 256], f, slc_ext[f],
                )

    _split_multi_wait_instructions(nc)
    return nc


def _get_graph(M):
    if M not in _GRAPH_CACHE:
        _GRAPH_CACHE[M] = _build_graph(M)
    return _GRAPH_CACHE[M]


# ---------------------------------------------------------------------------
# host side
# ---------------------------------------------------------------------------

def _build_mvt(ty, zero_out):
    """MvT[k, m] = Mv[m, k]; Mv[i, r]: (1-fy) at r=i+ay, fy at r=i+ay+1."""
    mvt = np.zeros((256, 256), dtype=np.float32)
    if zero_out:
        return mvt
    mty = -float(ty)
    ay = int(np.floor(mty))
    fy = mty - ay
    i = np.arange(256)
    r0 = i + ay
    v = (r0 >= 0) & (r0 < 256)
    mvt[r0[v], i[v]] = 1.0 - fy
    r1 = r0 + 1
    v1 = (r1 >= 0) & (r1 < 256)
    mvt[r1[v1], i[v1]] = fy
    return mvt


def kernel(x, dates, thetas):
    from concourse.bass_utils import run_bass_kernel_spmd

    x = np.ascontiguousarray(np.asarray(x, dtype=np.float32))
    dates = np.asarray(dates)
    thetas = np.asarray(thetas, dtype=np.float32)

    # reference frame selection (host: tiny)
    t_idx = np.argmin(np.abs(int(REF_DAY) - dates.astype(np.int64)), axis=1)

    # per-frame horizontal shift split: ax = floor(-tx), fx = frac
    txs = thetas[:, 0].astype(np.float64)
    axs = np.floor(-txs).astype(np.int64)
    fxs = (-txs) - axs
    maxax = int(np.abs(axs).max()) if axs.size else 0
    M = max(32, min(maxax, 258))
    M = (M + 1) // 2 * 2  # even

    nc = _get_graph(M)

    in_maps = []
    for c in range(NCORES):
        n = c // 2
        t0 = (c % 2) * FPC
        frames = slice(n * T + t0, n * T + t0 + FPC)

        xs = x[n, t0:t0 + FPC]                     # (12, 4, 256, 256)
        ref = np.ascontiguousarray(x[n, t_idx[n], 0:3])

        mvt = np.empty((FPC, 256, 256), dtype=np.float32)
        fracs = np.empty((2 * FPC,), dtype=np.float32)
        offs = np.empty((2 * FPC,), dtype=np.int32)
        for f in range(FPC):
            gi = n * T + t0 + f
            ax = int(axs[gi])
            fx = float(fxs[gi])
            horiz_ok = abs(ax) <= M
            mvt[f] = _build_mvt(thetas[gi, 1], zero_out=not horiz_ok)
            if not horiz_ok:
                ax, fx = 0, 0.0
            fracs[2 * f] = 1.0 - fx
            fracs[2 * f + 1] = fx
            offs[2 * f] = M + ax
            offs[2 * f + 1] = M + ax + 1

        gwc = np.zeros((40, 1), dtype=np.float32)
        for s in range(NF):
            gwc[3 * s:3 * s + 3, 0] = GRAY_W

        in_maps.append({
            "xs": np.ascontiguousarray(xs),
            "ref": ref,
            "mvt": np.ascontiguousarray(
                mvt.reshape(FPC, 2, 128, 256).transpose(2, 0, 1, 3)
            ).astype(ml_dtypes.bfloat16),
            "fracs": np.broadcast_to(fracs, (128, 2 * FPC)).copy(),
            "offs": offs.reshape(1, 2 * FPC),
            "ident": np.eye(128, dtype=np.float32),
            "gwc": gwc,
        })

    global _LAST_IN_MAPS
    _LAST_IN_MAPS = in_maps
    res = run_bass_kernel_spmd(nc, in_maps, list(range(NCORES)))

    out = np.empty((N, T, C, H, W), dtype=np.float32)
    x_slice = np.empty((N, T, H, W), dtype=np.float32)
    ref_gray = np.empty((N, H, W), dtype=np.float32)
    for c in range(NCORES):
        n = c // 2
        t0 = (c % 2) * FPC
        r = res.results[c]
        out[n, t0:t0 + FPC] = r["out"]
        x_slice[n, t0:t0 + FPC] = r["slc"]
        if t0 == 0:
            ref_gray[n] = r["rslc"]

    x_pairs = np.stack(
        [x_slice, np.broadcast_to(ref_gray[:, None], (N, T, H, W))], axis=2
    )
    return out, x_pairs
